# revision 11
# baseline (speedup 1.0000x reference)
"""BitLinear (1.58-bit) kernel for Trainium2, 8-core data-parallel SPMD.

Reference op: out = sign(x) @ ternarize(W).T where
  ternarize(W) = sign(W) * min(round(|W| / gamma), 1), gamma = mean(|W|) + 1e-6.

Strategy (per sharding hint: data-parallel over batch*seq, replicate ternary W):
  - Host: ternarize W once (the "small 2048x2048 ternary weight" of the hint),
    transpose to [in, out] and pack as fp8e4 (values -1/0/+1 are exact in fp8).
    Shard x by rows (batch*seq) across the 8 cores; pre-transpose each shard to
    [in, rows] so the contraction dim lands on SBUF partitions with contiguous
    DMA lines.
  - Device (per core): DMA x^T chunks (f32), compute sign() on the Scalar
    engine straight to fp8, then a dense fp8 DoubleRow matmul (2 MACs/cell/cyc)
    accumulating in PSUM f32.  Products are +-1 and row sums <= 2048 so fp32
    accumulation is exact.
  - Host: concatenate the 8 output shards.

Layout: contraction index i in [0, 2048) is split as i = kc*256 + j*128 + p
(kc = 256-wide chunk, j = DoubleRow pair slot, p = SBUF partition).  Both
operands are stored [128, KC, 2, N] in SBUF and sliced to the 3D
[128 part, 2, N] APs that MatmulPerfMode.DoubleRow requires.
"""

import numpy as np
import ml_dtypes

import concourse.bass as bass
import concourse.bacc as bacc
import concourse.mybir as mybir
from concourse.tile import TileContext
from concourse.bass_utils import run_bass_kernel_spmd

FP8 = ml_dtypes.float8_e4m3  # maps to mybir.dt.float8e4

N_CORES = 8
EPS = 1e-6

# Full-problem shapes (hardcoded per harness contract).
B, S, I_DIM, O_DIM = 4, 4096, 2048, 2048
M_TOT = B * S                 # 16384 rows
M_PER = M_TOT // N_CORES      # 2048 rows per core


def build_program(m_per: int, k_dim: int, o_dim: int) -> bass.Bass:
    """Per-core SPMD program: out[m, o] = sign(x)[m, :] @ Wq[o, :].T.

    DRAM inputs:
      xt : [KC, 128, 2, m_per] f32   (x^T, i = kc*256 + j*128 + p)
      wt : [KC, 128, 2, o_dim] fp8e4 (Wq^T, same i layout)
    DRAM output:
      out: [m_per, o_dim] f32
    """
    KC = k_dim // 256          # 256-wide contraction chunks
    MT = m_per // 128          # output row tiles
    OT = o_dim // 512          # output col chunks (one PSUM bank each)
    assert k_dim % 256 == 0 and m_per % 128 == 0 and o_dim % 512 == 0

    # Bacc (not plain Bass): its finalize() runs generate_event_semaphores,
    # which splits multi-waits to the HW limit of 1 wait per instruction.
    nc = bacc.Bacc()
    xt = nc.declare_dram_parameter(
        "xt", [KC, 128, 2, m_per], mybir.dt.bfloat16, isOutput=False)
    wt = nc.declare_dram_parameter(
        "wt", [KC, 128, 2, o_dim], mybir.dt.float8e4, isOutput=False)
    out = nc.declare_dram_parameter(
        "out", [m_per, o_dim], mybir.dt.float32, isOutput=True)

    with TileContext(nc) as tc:
        with (
            tc.tile_pool(name="wq", bufs=1) as wq_pool,
            tc.tile_pool(name="xs", bufs=1) as xs_pool,
            tc.tile_pool(name="xraw", bufs=1) as xraw_pool,
            tc.tile_pool(name="psum", bufs=8, space="PSUM") as psum_pool,
            tc.tile_pool(name="osb", bufs=3) as out_pool,
        ):
            # Quantized weight, fully SBUF-resident: 32 KB/partition (fp8).
            wq_sb = wq_pool.tile([128, KC, 2, o_dim], mybir.dt.float8e4)
            for kc in range(KC):
                nc.sync.dma_start(out=wq_sb[:, kc], in_=wt[kc])

            # x^T chunks: DMA bf16 (sign-exact), sign -> fp8 (Scalar engine).
            # Write-once staging (bufs=1, disjoint slices) keeps every HWDGE
            # DMA at <=2 sync waits (walrus limit).
            xr_sb = xraw_pool.tile([128, KC, 2, m_per], mybir.dt.bfloat16)
            xs_sb = xs_pool.tile([128, KC, 2, m_per], mybir.dt.float8e4)
            for kc in range(KC):
                nc.sync.dma_start(out=xr_sb[:, kc], in_=xt[kc])
                nc.scalar.activation(
                    out=xs_sb[:, kc], in_=xr_sb[:, kc],
                    func=mybir.ActivationFunctionType.Sign)

            # Dense fp8 DoubleRow matmul: lhsT = xs (stationary), rhs = wq.
            for mi in range(MT):
                psums = [psum_pool.tile([128, 512], mybir.dt.float32,
                                        name="ps", tag="ps")
                         for _ in range(OT)]
                for kc in range(KC):
                    lhsT = xs_sb[:, kc, :, bass.ts(mi, 128)]     # [128, 2, 128]
                    for oi in range(OT):
                        rhs = wq_sb[:, kc, :, bass.ts(oi, 512)]  # [128, 2, 512]
                        nc.tensor.matmul(
                            psums[oi], lhsT, rhs,
                            start=(kc == 0), stop=(kc == KC - 1),
                            perf_mode=mybir.MatmulPerfMode.DoubleRow)
                ot = out_pool.tile([128, o_dim], mybir.dt.float32, tag="ot")
                for oi in range(OT):
                    nc.any.tensor_copy(ot[:, bass.ts(oi, 512)], psums[oi])
                nc.scalar.dma_start(out=out[bass.ts(mi, 128)], in_=ot)

    # run_bass_via_pjrt does not finalize prebuilt modules; Bacc.finalize()
    # runs compile() (event-semaphore wait splitting, reg alloc, fusion).
    nc.finalize()
    return nc


def ternarize_host(weight: np.ndarray) -> np.ndarray:
    """absmean ternarization, f64 for a faithful gamma; returns {-1,0,1} f32."""
    w = weight.astype(np.float64)
    gamma = np.mean(np.abs(w)) + EPS
    return (np.sign(w) * np.minimum(np.round(np.abs(w) / gamma), 1.0)).astype(
        np.float32)


def _pack_kpj(a_t: np.ndarray) -> np.ndarray:
    """[k_dim, n] -> [KC, 128, 2, n] with i = kc*256 + j*128 + p."""
    k_dim, n = a_t.shape
    return np.ascontiguousarray(
        a_t.reshape(k_dim // 256, 2, 128, n).transpose(0, 2, 1, 3))


def prep_in_maps(x: np.ndarray, weight: np.ndarray) -> list[dict]:
    wq = ternarize_host(weight)                    # [o, i] ternary
    wt = _pack_kpj(np.ascontiguousarray(wq.T)).astype(FP8)  # [KC,128,2,o] fp8
    xf = x.reshape(M_TOT, I_DIM)
    in_maps = []
    for c in range(N_CORES):
        sh = xf[c * M_PER:(c + 1) * M_PER]         # [m_per, i]
        xt = _pack_kpj(np.ascontiguousarray(sh.T.astype(np.float32))).astype(
            ml_dtypes.bfloat16)  # bf16 is sign-exact for f32 normals
        in_maps.append({"xt": xt, "wt": wt})
    return in_maps


_PROGRAM_CACHE: dict = {}


def _get_program() -> bass.Bass:
    key = (M_PER, I_DIM, O_DIM)
    if key not in _PROGRAM_CACHE:
        _PROGRAM_CACHE[key] = build_program(*key)
    return _PROGRAM_CACHE[key]


def _gather(results: list[dict]) -> np.ndarray:
    full = np.concatenate([np.asarray(r["out"]) for r in results], axis=0)
    return np.ascontiguousarray(full.reshape(B, S, O_DIM).astype(np.float32))


def kernel(x: np.ndarray, weight: np.ndarray) -> np.ndarray:
    nc = _get_program()
    in_maps = prep_in_maps(np.asarray(x), np.asarray(weight))
    res = run_bass_kernel_spmd(nc, in_maps, core_ids=list(range(N_CORES)))
    return _gather(res.results)


def kernel_traced(x: np.ndarray, weight: np.ndarray, **trace_kw):
    """Like kernel() but returns (output, BassKernelResults) with a trace."""
    nc = _get_program()
    in_maps = prep_in_maps(np.asarray(x), np.asarray(weight))
    res = run_bass_kernel_spmd(
        nc, in_maps, core_ids=list(range(N_CORES)), trace=True, **trace_kw)
    return _gather(res.results), res


# revision 12
# speedup vs baseline: 1.0792x; 1.0792x over previous
"""BitLinear (1.58-bit) kernel for Trainium2, 8-core data-parallel SPMD.

Reference op: out = sign(x) @ ternarize(W).T where
  ternarize(W) = sign(W) * min(round(|W| / gamma), 1), gamma = mean(|W|) + 1e-6.

Strategy (per sharding hint: data-parallel over batch*seq, replicate ternary W):
  - Host: ternarize W once (the "small 2048x2048 ternary weight" of the hint),
    transpose to [in, out] and pack as fp8e4 (values -1/0/+1 are exact in fp8).
    Shard x by rows (batch*seq) across the 8 cores; pre-transpose each shard to
    [in, rows] so the contraction dim lands on SBUF partitions with contiguous
    DMA lines.
  - Device (per core): DMA x^T chunks (f32), compute sign() on the Scalar
    engine straight to fp8, then a dense fp8 DoubleRow matmul (2 MACs/cell/cyc)
    accumulating in PSUM f32.  Products are +-1 and row sums <= 2048 so fp32
    accumulation is exact.
  - Host: concatenate the 8 output shards.

Layout: contraction index i in [0, 2048) is split as i = kc*256 + j*128 + p
(kc = 256-wide chunk, j = DoubleRow pair slot, p = SBUF partition).  Both
operands are stored [128, KC, 2, N] in SBUF and sliced to the 3D
[128 part, 2, N] APs that MatmulPerfMode.DoubleRow requires.
"""

import numpy as np
import ml_dtypes

import concourse.bass as bass
import concourse.bacc as bacc
import concourse.mybir as mybir
from concourse.tile import TileContext
from concourse.bass_utils import run_bass_kernel_spmd

FP8 = ml_dtypes.float8_e4m3  # maps to mybir.dt.float8e4

N_CORES = 8
EPS = 1e-6

# Full-problem shapes (hardcoded per harness contract).
B, S, I_DIM, O_DIM = 4, 4096, 2048, 2048
M_TOT = B * S                 # 16384 rows
M_PER = M_TOT // N_CORES      # 2048 rows per core


def build_program(m_per: int, k_dim: int, o_dim: int) -> bass.Bass:
    """Per-core SPMD program: out[m, o] = sign(x)[m, :] @ Wq[o, :].T.

    DRAM inputs:
      xt : [KC, 128, 2, m_per] f32   (x^T, i = kc*256 + j*128 + p)
      wt : [KC, 128, 2, o_dim] fp8e4 (Wq^T, same i layout)
    DRAM output:
      out: [m_per, o_dim] f32
    """
    KC = k_dim // 256          # 256-wide contraction chunks
    MT = m_per // 128          # output row tiles
    OT = o_dim // 512          # output col chunks (one PSUM bank each)
    assert k_dim % 256 == 0 and m_per % 128 == 0 and o_dim % 512 == 0

    # Bacc (not plain Bass): its finalize() runs generate_event_semaphores,
    # which splits multi-waits to the HW limit of 1 wait per instruction.
    nc = bacc.Bacc()
    xt = nc.declare_dram_parameter(
        "xt", [KC, 128, 2, m_per], mybir.dt.bfloat16, isOutput=False)
    wt = nc.declare_dram_parameter(
        "wt", [KC, 128, 2, o_dim], mybir.dt.float8e4, isOutput=False)
    # f16 output: every value is an integer in [-2048, 2048], exact in f16;
    # the host casts back to f32.  Halves the output DMA traffic.
    out = nc.declare_dram_parameter(
        "out", [m_per, o_dim], mybir.dt.float16, isOutput=True)

    with TileContext(nc) as tc:
        with (
            tc.tile_pool(name="wq", bufs=1) as wq_pool,
            tc.tile_pool(name="xs", bufs=1) as xs_pool,
            tc.tile_pool(name="xraw", bufs=1) as xraw_pool,
            tc.tile_pool(name="psum", bufs=2, space="PSUM") as psum_pool,
            tc.tile_pool(name="osb", bufs=3) as out_pool,
        ):
            # x^T chunks first (PE's critical path at startup): DMA bf16
            # (sign-exact) on the SP queue, sign -> fp8 on the Scalar engine.
            # Write-once staging (bufs=1, disjoint slices) keeps every HWDGE
            # DMA at <=1 embedded sync wait (walrus limit).
            xr_sb = xraw_pool.tile([128, KC, 2, m_per], mybir.dt.bfloat16)
            xs_sb = xs_pool.tile([128, KC, 2, m_per], mybir.dt.float8e4)
            # Quantized weight, fully SBUF-resident: 32 KB/partition (fp8),
            # loaded on the ACT HWDGE queue in parallel with x on SP's.
            wq_sb = wq_pool.tile([128, KC, 2, o_dim], mybir.dt.float8e4)
            for kc in range(KC):
                nc.sync.dma_start(out=xr_sb[:, kc], in_=xt[kc])
                nc.scalar.dma_start(out=wq_sb[:, kc], in_=wt[kc])
                nc.scalar.activation(
                    out=xs_sb[:, kc], in_=xr_sb[:, kc],
                    func=mybir.ActivationFunctionType.Sign)

            # Dense fp8 DoubleRow matmul: lhsT = xs (stationary), rhs = wq.
            # One 4-bank PSUM tile per mi; matmuls write bank-aligned slices.
            for mi in range(MT):
                ps = psum_pool.tile([128, OT * 512], mybir.dt.float32,
                                    name="ps", tag="ps")
                for kc in range(KC):
                    lhsT = xs_sb[:, kc, :, bass.ts(mi, 128)]     # [128, 2, 128]
                    for oi in range(OT):
                        rhs = wq_sb[:, kc, :, bass.ts(oi, 512)]  # [128, 2, 512]
                        nc.tensor.matmul(
                            ps[:, bass.ts(oi, 512)], lhsT, rhs,
                            start=(kc == 0), stop=(kc == KC - 1),
                            perf_mode=mybir.MatmulPerfMode.DoubleRow)
                # psum -> sbuf (f32 -> f16, exact) split across DVE and ACT,
                # then one 0.5 MB DMA out on the ACT HWDGE queue.
                ot = out_pool.tile([128, o_dim], mybir.dt.float16, tag="ot")
                half = OT * 512 // 2
                nc.vector.tensor_copy(ot[:, :half], ps[:, :half])
                nc.scalar.copy(ot[:, half:], ps[:, half:])
                nc.scalar.dma_start(out=out[bass.ts(mi, 128)], in_=ot)

    # run_bass_via_pjrt does not finalize prebuilt modules; Bacc.finalize()
    # runs compile() (event-semaphore wait splitting, reg alloc, fusion).
    nc.finalize()
    return nc


def ternarize_host(weight: np.ndarray) -> np.ndarray:
    """absmean ternarization, f64 for a faithful gamma; returns {-1,0,1} f32."""
    w = weight.astype(np.float64)
    gamma = np.mean(np.abs(w)) + EPS
    return (np.sign(w) * np.minimum(np.round(np.abs(w) / gamma), 1.0)).astype(
        np.float32)


def _pack_kpj(a_t: np.ndarray) -> np.ndarray:
    """[k_dim, n] -> [KC, 128, 2, n] with i = kc*256 + j*128 + p."""
    k_dim, n = a_t.shape
    return np.ascontiguousarray(
        a_t.reshape(k_dim // 256, 2, 128, n).transpose(0, 2, 1, 3))


def prep_in_maps(x: np.ndarray, weight: np.ndarray) -> list[dict]:
    wq = ternarize_host(weight)                    # [o, i] ternary
    wt = _pack_kpj(np.ascontiguousarray(wq.T)).astype(FP8)  # [KC,128,2,o] fp8
    xf = x.reshape(M_TOT, I_DIM)
    in_maps = []
    for c in range(N_CORES):
        sh = xf[c * M_PER:(c + 1) * M_PER]         # [m_per, i]
        xt = _pack_kpj(np.ascontiguousarray(sh.T.astype(np.float32))).astype(
            ml_dtypes.bfloat16)  # bf16 is sign-exact for f32 normals
        in_maps.append({"xt": xt, "wt": wt})
    return in_maps


_PROGRAM_CACHE: dict = {}


def _get_program() -> bass.Bass:
    key = (M_PER, I_DIM, O_DIM)
    if key not in _PROGRAM_CACHE:
        _PROGRAM_CACHE[key] = build_program(*key)
    return _PROGRAM_CACHE[key]


def _gather(results: list[dict]) -> np.ndarray:
    full = np.concatenate([np.asarray(r["out"]) for r in results], axis=0)
    return np.ascontiguousarray(full.reshape(B, S, O_DIM).astype(np.float32))


def kernel(x: np.ndarray, weight: np.ndarray) -> np.ndarray:
    nc = _get_program()
    in_maps = prep_in_maps(np.asarray(x), np.asarray(weight))
    res = run_bass_kernel_spmd(nc, in_maps, core_ids=list(range(N_CORES)))
    return _gather(res.results)


def kernel_traced(x: np.ndarray, weight: np.ndarray, **trace_kw):
    """Like kernel() but returns (output, BassKernelResults) with a trace."""
    nc = _get_program()
    in_maps = prep_in_maps(np.asarray(x), np.asarray(weight))
    res = run_bass_kernel_spmd(
        nc, in_maps, core_ids=list(range(N_CORES)), trace=True, **trace_kw)
    return _gather(res.results), res
